# revision 57
# baseline (speedup 1.0000x reference)
"""BitLinearOptimized Trainium2 kernel — 8-core SPMD, self-contained.

kernel(**inputs) takes the FULL inputs (input [8192,4096] f32,
weight [4096,4096] f32 ternary, weight_scale [1] f32, bias [4096] f32)
and returns the FULL output [8192, 4096] f32.

Sharding: input row-sharded 8 ways (each core quantizes its rows),
weight sharded along out_features (each core group-sums its shard, then
AllGathers the small reduced w_sumT so every core holds all out
features). A global absmax AllReduce(max) provides act_scale. Each core
computes outT[:, its rows] = w_sumT.T @ x_sumT (f16 operands, fp32
PSUM — exact integer arithmetic), applies scale+bias, host concatenates.

v7 notes (all from measured traces):
- per-ring DMA ~73GB/s, ~220GB/s aggregate; x first on all three rings
  (a few w chunks interleaved so the w path finishes just in time for
  its AllGather), the rest of w behind x with small per-ring pools so
  descriptor-gen never couples rings together through a shared pool.
- collective triggers block their queue while waiting on input sems
  (and Tile may reorder them ahead of other queue entries): the gpsimd
  queue carries only DMAs that are safe to wait, and the gmax readback
  lives on the scalar ring (collective output -> reader deps are
  tracked).
- scalars are broadcast to all partitions via a tiny K=1 PE matmul
  (ones x [recip, sc]); stride-0 raw-AP DMA bounces are INVISIBLE to
  the dependency tracker and race (proven by partial-tile corruption).
- the CC core serializes meshes in emission order: AR then AG; each
  mesh costs ~15-40us of peer-wait/hop latency regardless of payload.
- quantize: DVE in-place magic-round (2x mode), Act contiguous
  magic-subtract to f16 (strided writes are 4.5x slower - avoid),
  pairwise group adds on DVE (stride-4 reads, final add 2x packed).
- outputs written f16 (math otherwise exact, ~3e-4 rel err vs 2e-2
  budget); stationary loads own sync; outs split sync/scalar.
- gpsimd Q7 does no bulk elementwise work (~15x slower than DVE/Act).
"""

import numpy as np

import concourse.bass as bass
from concourse import bacc
import concourse.mybir as mybir
import concourse.tile as tile

F32 = mybir.dt.float32
F16 = mybir.dt.float16
MAGIC_C = float(np.float32(1.5 * 2**23))

# problem shape (hardcoded per contest contract)
N_FULL, IN_F, OUT_F, NCORES = 8192, 4096, 4096, 8


def build_bitlinear(N=N_FULL, IN=IN_F, OUT=OUT_F, ncores=NCORES):
    P = 128
    ROWS = N // ncores          # rows per core (1024)
    OCOLS = OUT // ncores       # out features per core (512)
    G = IN // 4                 # groups (1024)
    RT = ROWS // P              # row tiles (8)
    GT = G // P                 # k tiles for matmul (8)
    WT = OCOLS // P             # w shard row tiles (4)
    NCH = 512                   # matmul moving free dim
    NNT = ROWS // NCH           # row chunks (2)
    HC = IN // 2                # quantize half-tile (2048)
    GH = G // 2                 # groups per half (512)
    WCH = 512                   # w load chunk (free dim)
    WCT = IN // WCH             # w chunks per w row tile (8)
    NWC = WT * WCT              # total w chunks (32)
    assert ROWS % P == 0 and G % P == 0 and OCOLS % P == 0

    core_ids = list(range(ncores))
    nc = bacc.Bacc(num_devices=ncores)

    x_d = nc.declare_dram_parameter("x_loc", [ROWS, IN], F32, isOutput=False)
    w_d = nc.declare_dram_parameter("w_loc", [OCOLS, IN], F32, isOutput=False)
    ws_d = nc.declare_dram_parameter("wscale", [1, 1], F32, isOutput=False)
    bias_d = nc.declare_dram_parameter("bias", [OUT], F32, isOutput=False)
    outT_d = nc.declare_dram_parameter("outT", [OUT, ROWS], F16, isOutput=True)

    # collective bounce buffers (internal DRAM; outputs Shared)
    mxb_d = nc.dram_tensor("mx_bounce", [128], F32)
    ar_in_d = nc.dram_tensor("ar_in", [128], F32)
    ar_out_d = nc.dram_tensor("ar_out", [128], F32, addr_space="Shared")
    # partition-major reduced weights: [p, wt, k, o]; a remote block is
    # 8KB contiguous per partition.
    wsT_loc_d = nc.dram_tensor("wsT_loc", [P, WT * GT * P], F16)
    wsT_all_d = nc.dram_tensor("wsT_all", [ncores * P, WT * GT * P], F16,
                               addr_space="Shared")

    with tile.TileContext(nc) as tc:
        with (
            tc.tile_pool(name="xp", bufs=RT) as xp,           # resident x
            tc.tile_pool(name="qp", bufs=2) as qp,             # q half-tiles
            tc.tile_pool(name="qab", bufs=2) as qabp,          # pair sums
            tc.tile_pool(name="xsum", bufs=2) as xsump,        # xs [P, G]
            tc.tile_pool(name="wlds", bufs=3) as wldsp,        # w chunks sync
            tc.tile_pool(name="wlda", bufs=3) as wldap,        # w chunks act
            tc.tile_pool(name="wldg", bufs=2) as wldgp,        # w chunks gp
            tc.tile_pool(name="wsum", bufs=2) as wsump,
            tc.tile_pool(name="w3T", bufs=2) as w3Tp,          # w transposed
            tc.tile_pool(name="xsT", bufs=1) as xsTp,
            tc.tile_pool(name="wst", bufs=2) as wstp,          # stationary
            tc.tile_pool(name="outp", bufs=3) as outp,
            tc.tile_pool(name="cst", bufs=1) as cst,
            tc.tile_pool(name="ps", bufs=7, space="PSUM") as psp,
            tc.tile_pool(name="psb", bufs=1, space="PSUM") as psbp,
        ):
            # w chunk ring assignment and pools
            wpool = {0: wldsp, 1: wldap, 2: wldgp}
            weng = {0: nc.sync, 1: nc.scalar, 2: nc.gpsimd}
            # chunk index -> ring: first six (the early ones) round-robin,
            # the rest alternate sync/scalar only
            wring = [c % 3 if c < 6 else c % 2 for c in range(NWC)]
            wla = [None] * NWC

            def load_w(c):
                wt, ck = c // WCT, c % WCT
                r = wring[c]
                wl = wpool[r].tile([P, WCH], F32, tag=f"wld{r}")
                weng[r].dma_start(
                    out=wl[:], in_=w_d[wt * P:(wt + 1) * P,
                                       ck * WCH:(ck + 1) * WCH])
                wla[c] = wl

            # ---------------- phase A: x loads on all three rings ------------
            # two w chunks per ring slotted before the last x tile so the w
            # path finishes just after x
            x_eng = [nc.sync, nc.scalar, nc.gpsimd]
            xta = []
            for rt in range(RT):
                xt = xp.tile([P, IN], F32, tag="xp", name=f"x{rt}")
                x_eng[rt % 3].dma_start(out=xt[:], in_=x_d[rt * P:(rt + 1) * P, :])
                xta.append(xt)
                if rt == 4:
                    for c in range(6):
                        load_w(c)

            # weight_scale scalar + all-ones row for the PE broadcast trick
            ws_sb = cst.tile([1, 1], F32, tag="ws_sb")
            nc.scalar.dma_start(out=ws_sb[:], in_=ws_d[:])
            ones1 = cst.tile([1, P], F32, tag="ones1")
            nc.vector.memset(ones1[:], 1.0)
            bias_sb = cst.tile([P, OUT // P], F32, tag="bias_sb")
            nc.scalar.dma_start(out=bias_sb[:],
                                in_=bias_d[:].rearrange("(b p) -> p b", p=P))

            # local absmax -> [128] vector -> one DMA -> AllReduce(max).
            # Keep the pre-trigger path minimal (one DMA): trigger-time skew
            # across cores feeds straight into the mesh's peer-wait.
            mxcol = cst.tile([P, RT], F32, tag="mxcol")
            for rt in range(RT):
                nc.vector.tensor_reduce(out=mxcol[:, rt:rt + 1], in_=xta[rt][:],
                                        axis=mybir.AxisListType.X,
                                        op=mybir.AluOpType.max,
                                        apply_absolute_value=True)
            mx1 = cst.tile([P, 1], F32, tag="mx1")
            nc.vector.tensor_reduce(out=mx1[:], in_=mxcol[:],
                                    axis=mybir.AxisListType.X,
                                    op=mybir.AluOpType.max)
            # The whole AllReduce + scalar chain lives on the gpsimd SWDGE
            # queue (single queue -> FIFO-ordered DMAs; the collective
            # trigger blocks the queue until the mesh completes). Raw-AP
            # stride-0 broadcasts on the multi-queue HWDGE rings are racy.
            nc.gpsimd.dma_start(out=ar_in_d[:].rearrange("(p s) -> p s", p=P),
                                in_=mx1[:])
            nc.gpsimd.collective_compute(
                "AllReduce", mybir.AluOpType.max,
                replica_groups=[core_ids],
                ins=[ar_in_d[:]], outs=[ar_out_d[:]],
            )
            # gmax read on the scalar ring: collective output -> reader deps
            # ARE tracked; the gpsimd queue must stay clear because Tile can
            # reorder the (blocking) AllGather trigger ahead of anything
            # placed there.
            gmax = cst.tile([1, P], F32, tag="gmax")
            nc.scalar.dma_start(out=gmax[:],
                                in_=ar_out_d[:].rearrange("(a b) -> a b", a=1))
            mloc = cst.tile([1, 1], F32, tag="mloc")
            nc.vector.tensor_reduce(out=mloc[:], in_=gmax[:],
                                    axis=mybir.AxisListType.X,
                                    op=mybir.AluOpType.max)
            # act_scale = gmax/127; recip = 1/act_scale; sc = ws*act_scale/4
            asc = cst.tile([1, 1], F32, tag="asc")
            nc.vector.tensor_scalar(out=asc[:], in0=mloc[0:1, 0:1],
                                    scalar1=float(np.float32(1.0 / 127.0)),
                                    scalar2=None,
                                    op0=mybir.AluOpType.mult)
            rec1 = cst.tile([1, 1], F32, tag="rec1")
            nc.vector.reciprocal(out=rec1[:], in_=asc[:])
            sc1 = cst.tile([1, 1], F32, tag="sc1")
            nc.vector.tensor_tensor(out=sc1[:], in0=ws_sb[:], in1=asc[:],
                                    op=mybir.AluOpType.mult)
            nc.vector.tensor_scalar(out=sc1[:], in0=sc1[:], scalar1=0.25,
                                    scalar2=None, op0=mybir.AluOpType.mult)
            sc2 = cst.tile([1, 2], F32, tag="sc2")
            nc.vector.tensor_copy(out=sc2[0:1, 0:1], in_=rec1[:])
            nc.vector.tensor_copy(out=sc2[0:1, 1:2], in_=sc1[:])
            # broadcast (recip, sc) to all 128 partitions through the PE:
            # out[p, j] = ones[0, p] * sc2[0, j]. Fully dependency-tracked,
            # unlike a stride-0 raw-AP DMA bounce (those are invisible to
            # the Tile dependency tracker and race).
            psb = psbp.tile([P, 2], F32, tag="psb")
            nc.tensor.matmul(psb[:], lhsT=ones1[:], rhs=sc2[:],
                             start=True, stop=True)
            scbc = cst.tile([P, 2], F32, tag="scbc")
            nc.vector.tensor_copy(out=scbc[:], in_=psb[:])
            recip = scbc[:, 0:1]
            sc = scbc[:, 1:2]

            # remaining w chunks (behind x, sync+scalar rings only: the
            # gpsimd queue must stay clear for the collective chain)
            for c in range(6, NWC):
                load_w(c)

            # ---------------- w path: group-sum + transpose ------------------
            with nc.allow_low_precision(reason="w_sum in [-4,4], exact in f16"):
                wsums = []
                for wt in range(WT):
                    wsum_t = wsump.tile([P, G], F16, tag="wsum")
                    for ck in range(WCT):
                        gch = WCH // 4
                        nc.vector.tensor_reduce(
                            out=wsum_t[:, ck * gch:(ck + 1) * gch],
                            in_=wla[wt * WCT + ck][:]
                                .rearrange("p (g f) -> p g f", f=4),
                            axis=mybir.AxisListType.X,
                            op=mybir.AluOpType.add)
                    wsums.append(wsum_t)
            for wt in range(WT):
                w3T = w3Tp.tile([P, GT, P], F16, tag="w3T", name=f"w3T{wt}")
                nc.scalar.dma_start_transpose(w3T[:], wsums[wt][:])
                nc.sync.dma_start(
                    out=wsT_loc_d[:, wt * GT * P:(wt + 1) * GT * P]
                        .rearrange("p (a o) -> p a o", a=GT),
                    in_=w3T[:])
            nc.gpsimd.collective_compute(
                "AllGather", mybir.AluOpType.bypass,
                replica_groups=[core_ids],
                ins=[wsT_loc_d[:]], outs=[wsT_all_d[:]],
            )

            # ---------------- quantize (in place) + group-sum + transpose ---
            xsT3 = xsTp.tile([P, GT, ROWS], F16, tag="xsT3")
            with nc.allow_low_precision(reason="x_q sums <=508, exact in f16"):
                for rt in range(RT):
                    xt = xta[rt]
                    nc.vector.tensor_scalar(out=xt[:], in0=xt[:],
                                            scalar1=recip, scalar2=MAGIC_C,
                                            op0=mybir.AluOpType.mult,
                                            op1=mybir.AluOpType.add)
                    xs = xsump.tile([P, G], F16, tag="xsum")
                    for h in range(2):
                        qh = qp.tile([P, HC], F16, tag="qp")
                        nc.scalar.activation(
                            out=qh[:], in_=xt[:, h * HC:(h + 1) * HC],
                            func=mybir.ActivationFunctionType.Copy,
                            bias=-MAGIC_C, scale=1.0)
                        q3 = qh[:].rearrange("p (g f) -> p g f", f=4)
                        qa = qabp.tile([P, GH], F16, tag="qab")
                        qb = qabp.tile([P, GH], F16, tag="qab")
                        nc.vector.tensor_tensor(out=qa[:], in0=q3[:, :, 0],
                                                in1=q3[:, :, 1],
                                                op=mybir.AluOpType.add)
                        nc.vector.tensor_tensor(out=qb[:], in0=q3[:, :, 2],
                                                in1=q3[:, :, 3],
                                                op=mybir.AluOpType.add)
                        nc.vector.tensor_tensor(
                            out=xs[:, h * GH:(h + 1) * GH],
                            in0=qa[:], in1=qb[:], op=mybir.AluOpType.add)
                    treng = nc.sync if rt in (1, 3) else nc.scalar
                    treng.dma_start_transpose(
                        xsT3[:, :, rt * P:(rt + 1) * P], xs[:])

            # ---------------- matmul + epilogue ------------------------------
            # rblk outer: each gathered stationary block loaded once (sync
            # ring); f16 outputs split across both rings.
            HB = WT * GT * P // 2
            for rblk in range(ncores):
                wst = wstp.tile([P, WT * GT * P], F16, tag="wst",
                                name=f"wst{rblk}")
                # halves on two rings so the stationary stream outpaces the PE
                nc.sync.dma_start(
                    out=wst[:, 0:HB],
                    in_=wsT_all_d[rblk * P:(rblk + 1) * P, 0:HB])
                nc.gpsimd.dma_start(
                    out=wst[:, HB:],
                    in_=wsT_all_d[rblk * P:(rblk + 1) * P, HB:])
                w4 = wst[:].rearrange("p (w a o) -> p w a o", w=WT, a=GT)
                for wt in range(WT):
                    for nn in range(NNT):
                        ps = psp.tile([P, NCH], F32, tag="ps",
                                      name=f"ps{rblk}_{wt}_{nn}")
                        for k in range(GT):
                            nc.tensor.matmul(
                                ps[:],
                                lhsT=w4[:, wt, k, :],
                                rhs=xsT3[:, k, nn * NCH:(nn + 1) * NCH],
                                start=(k == 0), stop=(k == GT - 1))
                        ob = rblk * WT + wt
                        ot = outp.tile([P, NCH], F16, tag="ot")
                        if (wt + nn) % 2 == 0:
                            nc.scalar.activation(
                                out=ot[:], in_=ps[:],
                                func=mybir.ActivationFunctionType.Identity,
                                scale=sc,
                                bias=bias_sb[:, ob:ob + 1])
                        else:
                            nc.vector.tensor_scalar(
                                out=ot[:], in0=ps[:],
                                scalar1=sc,
                                scalar2=bias_sb[:, ob:ob + 1],
                                op0=mybir.AluOpType.mult,
                                op1=mybir.AluOpType.add)
                        oeng = nc.scalar if (wt + nn) % 2 == 0 else nc.sync
                        oeng.dma_start(
                            out=outT_d[ob * P:(ob + 1) * P,
                                       nn * NCH:(nn + 1) * NCH],
                            in_=ot[:])

    return nc


def make_in_maps(inputs, ncores=NCORES):
    x = np.ascontiguousarray(np.asarray(inputs["input"], dtype=np.float32))
    w = np.ascontiguousarray(np.asarray(inputs["weight"], dtype=np.float32))
    ws = np.asarray(inputs["weight_scale"], dtype=np.float32).reshape(1, 1)
    b = np.ascontiguousarray(np.asarray(inputs["bias"], dtype=np.float32))
    N = x.shape[0]
    OUT = w.shape[0]
    ROWS = N // ncores
    OCOLS = OUT // ncores
    return [
        {
            "x_loc": x[c * ROWS:(c + 1) * ROWS],
            "w_loc": w[c * OCOLS:(c + 1) * OCOLS],
            "wscale": ws,
            "bias": b,
        }
        for c in range(ncores)
    ]


def assemble_output(results):
    return np.ascontiguousarray(
        np.concatenate(
            [np.asarray(r["outT"]).astype(np.float32).T for r in results],
            axis=0))


_NC_CACHE = {}


def _get_nc():
    key = (N_FULL, IN_F, OUT_F, NCORES)
    if key not in _NC_CACHE:
        nc = build_bitlinear(*key)
        if not nc.is_finalized():
            nc.finalize()
        _NC_CACHE[key] = nc
    return _NC_CACHE[key]


def run_on_hw(inputs, trace=False):
    from concourse.bass_utils import run_bass_kernel_spmd
    nc = _get_nc()
    in_maps = make_in_maps(inputs)
    res = run_bass_kernel_spmd(nc, in_maps, list(range(NCORES)), trace=trace)
    return assemble_output(res.results), res


def kernel(**inputs) -> np.ndarray:
    out, _ = run_on_hw(inputs, trace=False)
    return out
